# revision 43
# baseline (speedup 1.0000x reference)
"""Trainium2 Bass kernel for nn_Bottleneck (sparse 3x3 gather-GEMM bottleneck block).

Strategy (8 NeuronCores, zero cross-core communication):
  - Connected components of the 19%-occupancy Moore-stencil graph are tiny;
    host assigns whole components to cores so every neighbor is core-local.
  - Channel-major compute; LN centering folded into conv weights; variance
    via ones-matmul; rstd broadcast via small DRAM-bounce DMAs; LN3 variance
    before conv3 via a Cholesky factor of W3c W3c^T / 256.
  - conv2 is EDGE-COMPACTED: only real edges per tap (~4.9k vs 25.6k) are
    gathered from a token-major table, multiplied in a flipped matmul
    (lhsT = gathered activations -> token-major products) and
    dma_scatter_add-ed (bf16) into SBUF parity accumulators. Center tap is a
    dense matmul over h1. Edges are split by destination half (lo/hi) so the
    epilogue for lo-tokens overlaps the hi-half scatter work (the Q7
    descriptor generator is the critical resource).
  - Table built with PE transposes (tensor engine) instead of DMA transposes.
  - x and y are bf16 in DRAM (host converts) to halve HBM traffic.
"""

import os
import sys

sys.path.insert(0, "/opt/trn_rl_repo")

import numpy as np

import concourse.bass as bass
import concourse.tile as tile
from concourse import bacc as bacc_mod
from concourse import library_config, mybir
from concourse.bass_utils import run_bass_kernel_spmd

# Problem constants (hardcoded per contract).
N = 200000
C_IN = 256
C_MID = 64
EPS = 1e-6
NCORES = 8

T = 25600
PAIR = 1024
SUB = 512
NPAIR = T // PAIR          # 25
NGRP = (NPAIR + 1) // 2    # 13 groups of <=2 pairs
NBLK = T // 128            # 200 table blocks
SENT = T                   # sentinel token id -> zeroed table rank
NRANK = NBLK + 1           # 201 ranks in the gather table
KS8 = [0, 1, 2, 3, 5, 6, 7, 8]  # non-center taps
KTAP = [0, 1, 2, 6, 7, 8, 3, 5]  # emit order: 6 gathered taps + 2 boundary-minis
NMINI = 128

SPLIT = 12288              # dst-half boundary (pair 12), 96 blocks
NKLO = 2432                # padded lo edges per tap (max measured 2420)
NKHI = 2560                # padded hi edges per tap (max measured 2529)
NCHLO = NKLO // 128        # 20
NCHHI = NKHI // 128        # 21
NGLO = SPLIT // 256 + 1    # 49 groups (48 real + trash)
NGHI = (T - SPLIT) // 256 + 1  # 53 groups (52 real + trash)
LO_TRASH = SPLIT           # -> group 48
HI_TRASH = T - SPLIT       # rebased -> group 52

f32 = mybir.dt.float32
f32r = mybir.dt.float32r
bf16 = mybir.dt.bfloat16
i16 = mybir.dt.int16
AF = mybir.ActivationFunctionType
OP = mybir.AluOpType

_NC_CACHE = {}


def _bcast_ap(src: bass.AP, ap_dims):
    """Manual access pattern with explicit [step, count] dims over src."""
    return bass.AP(tensor=src.tensor, offset=src.offset, ap=ap_dims)


def build_nc():
    if "nc" in _NC_CACHE:
        return _NC_CACHE["nc"]
    nc = bacc_mod.Bacc(None, target_bir_lowering=False, debug=False)

    x_t = nc.declare_dram_parameter("x_t", [2, 128, T], bf16, isOutput=False)
    w1 = nc.declare_dram_parameter("w1", [128, 2, C_MID], bf16, isOutput=False)
    w2 = nc.declare_dram_parameter("w2", [C_MID, 9, C_MID], bf16, isOutput=False)
    w2c = nc.declare_dram_parameter("w2c", [128, C_MID], bf16, isOutput=False)
    w2d = nc.declare_dram_parameter("w2d", [128, 2, C_MID], bf16, isOutput=False)
    w3 = nc.declare_dram_parameter("w3", [128, C_IN], bf16, isOutput=False)
    lmat = nc.declare_dram_parameter("lmat", [128, C_MID], bf16, isOutput=False)
    es = nc.declare_dram_parameter("es", [128, 2, 4], f32r, isOutput=False)
    g1r = nc.declare_dram_parameter("g1r", [128, 1], f32, isOutput=False)
    b1r = nc.declare_dram_parameter("b1r", [128, 1], f32, isOutput=False)
    g2r = nc.declare_dram_parameter("g2r", [128, 1], f32, isOutput=False)
    b2r = nc.declare_dram_parameter("b2r", [128, 1], f32, isOutput=False)
    g3r = nc.declare_dram_parameter("g3r", [128, 2], f32, isOutput=False)
    b3r = nc.declare_dram_parameter("b3r", [128, 2], f32, isOutput=False)
    identb = nc.declare_dram_parameter("identb", [128, 128], bf16, isOutput=False)
    gidx = nc.declare_dram_parameter(
        "gidx", [8, 128, (NKLO + NKHI) // 16], i16, isOutput=False
    )
    sidx = nc.declare_dram_parameter(
        "sidx", [8, 128, (NKLO + NKHI) // 16], i16, isOutput=False
    )
    mask_d = nc.declare_dram_parameter(
        "mask_d", [128, NPAIR, 2, SUB], bf16, isOutput=False
    )
    y_t = nc.declare_dram_parameter("y_t", [2, 128, T], bf16, isOutput=True)
    dbg = os.environ.get("KDBG", "0") == "1"
    if dbg:
        acclo_out = nc.declare_dram_parameter(
            "acclo_out", [128, 2, NGLO, C_MID], bf16, isOutput=True
        )
        acchi_out = nc.declare_dram_parameter(
            "acchi_out", [128, 2, NGHI, C_MID], bf16, isOutput=True
        )

    from contextlib import ExitStack

    with ExitStack() as ctx:
        tc = ctx.enter_context(tile.TileContext(nc))
        consts = ctx.enter_context(tc.tile_pool(name="consts", bufs=1))
        tablep = ctx.enter_context(tc.tile_pool(name="table", bufs=1))
        h1p = ctx.enter_context(tc.tile_pool(name="h1cm", bufs=1))
        accp = ctx.enter_context(tc.tile_pool(name="acc", bufs=1))
        xp = ctx.enter_context(tc.tile_pool(name="xin", bufs=2))
        sqp = ctx.enter_context(tc.tile_pool(name="sq", bufs=2))
        rsp = ctx.enter_context(tc.tile_pool(name="rstd", bufs=2))
        rbp = ctx.enter_context(tc.tile_pool(name="rbc", bufs=2))
        drp = ctx.enter_context(tc.tile_pool(name="drscratch", bufs=2, space="DRAM"))

        # ---- constants into SBUF ----
        w1_sb = consts.tile([128, 2, C_MID], bf16)
        nc.sync.dma_start(out=w1_sb[:], in_=w1[:])
        w2_sb = consts.tile([C_MID, 9, C_MID], bf16)
        nc.sync.dma_start(out=w2_sb[:], in_=w2[:])
        w2c_sb = consts.tile([128, C_MID], bf16)
        nc.sync.dma_start(out=w2c_sb[:], in_=w2c[:])
        w2d_sb = consts.tile([128, 2, C_MID], bf16)
        nc.sync.dma_start(out=w2d_sb[:], in_=w2d[:])
        w3_sb = consts.tile([128, C_IN], bf16)
        nc.sync.dma_start(out=w3_sb[:], in_=w3[:])
        l_sb = consts.tile([128, C_MID], bf16)
        nc.sync.dma_start(out=l_sb[:], in_=lmat[:])
        es_sb = consts.tile([128, 2, 4], f32r)
        nc.sync.dma_start(out=es_sb[:], in_=es[:])
        esb_sb = consts.tile([128, 2, 4], bf16)
        nc.gpsimd.dma_start(out=esb_sb[:], in_=es[:].bitcast(f32))
        identb_sb = consts.tile([128, 128], bf16)
        nc.sync.dma_start(out=identb_sb[:], in_=identb[:])
        sc = {}
        for name, src in (("g1", g1r), ("b1", b1r), ("g2", g2r), ("b2", b2r)):
            t_ = consts.tile([128, 1], f32, tag=f"sc_{name}")
            nc.sync.dma_start(out=t_[:], in_=src[:])
            sc[name] = t_
        g3_sb = consts.tile([128, 2], f32)
        nc.sync.dma_start(out=g3_sb[:], in_=g3r[:])
        b3_sb = consts.tile([128, 2], f32)
        nc.sync.dma_start(out=b3_sb[:], in_=b3r[:])
        eps_sb = consts.tile([128, 1], f32)
        nc.vector.memset(eps_sb[:], EPS)
        ones_sb = consts.tile([128, 1], f32)
        nc.vector.memset(ones_sb[:], 1.0)

        table = tablep.tile([128, NRANK * 128], bf16)   # point-major gather table
        nc.vector.memset(table[:], 0.0)
        h1cm = h1p.tile([128, NPAIR * SUB], bf16)       # channel-major h1n

        def pairs_of(g):
            return [p for p in (2 * g, 2 * g + 1) if p < NPAIR]

        # =================== PHASE A: conv1 + LN1 + GELU -> table ============
        with tc.tile_pool(name="psA", bufs=4, space="PSUM") as psA, \
             tc.tile_pool(name="psGA", bufs=2, space="PSUM") as psGA, \
             tc.tile_pool(name="psTab", bufs=2, space="PSUM") as psTabp:
            for g in range(NGRP):
                ps = pairs_of(g)
                gridA = psGA.tile([128, SUB], f32, tag="grid")
                if len(ps) == 1:
                    nc.vector.memset(gridA[0:4, :], 1.0)
                a1s = {}
                for j, p in enumerate(ps):
                    x_sb = xp.tile([128, 2, PAIR], bf16, tag="xin")
                    nc.scalar.dma_start(
                        out=x_sb[:],
                        in_=x_t[:, :, p * PAIR:(p + 1) * PAIR].rearrange(
                            "c p f -> p c f"
                        ),
                    )
                    a1 = psA.tile([128, SUB], f32, tag="A1")
                    a1s[j] = a1
                    for s in range(2):
                        for c in range(2):
                            nc.tensor.matmul(
                                out=a1[64 * s:64 * s + 64, :],
                                lhsT=w1_sb[:, c, :],
                                rhs=x_sb[:, c, SUB * s:SUB * (s + 1)],
                                start=(c == 0),
                                stop=(c == 1),
                                tile_position=(0, 64 * s),
                            )
                    sq = sqp.tile([128, SUB], f32r, tag="sq")
                    nc.scalar.activation(out=sq[:], in_=a1[:], func=AF.Square)
                    nc.tensor.matmul(
                        out=gridA[0:4, :],
                        lhsT=es_sb[:, j, :],
                        rhs=sq[:],
                        start=(j == 0),
                        stop=(j == len(ps) - 1),
                    )
                rstd = rsp.tile([4, SUB], f32, tag="rstd")
                nc.scalar.activation(
                    out=rstd[:], in_=gridA[0:4, :], func=AF.Sqrt,
                    bias=eps_sb[0:4, :], scale=1.0 / C_MID,
                )
                nc.vector.reciprocal(out=rstd[:], in_=rstd[:])
                rsc = drp.tile([4, SUB], f32, tag="rsc")
                nc.sync.dma_start(out=rsc[:], in_=rstd[:])
                rb = rbp.tile([128, 2, SUB], f32, tag="rb1")
                for j in range(len(ps)):
                    for s in range(2):
                        nc.sync.dma_start(
                            out=rb[64 * s:64 * s + 64, j, :],
                            in_=_bcast_ap(
                                rsc[2 * j + s:2 * j + s + 1, :],
                                [[0, 64], [1, SUB]],
                            ),
                        )
                for j, p in enumerate(ps):
                    h1s = h1cm[:, p * SUB:(p + 1) * SUB]
                    nc.vector.scalar_tensor_tensor(
                        out=h1s, in0=a1s[j][:], scalar=sc["g1"][:],
                        in1=rb[:, j, :], op0=OP.mult, op1=OP.mult,
                    )
                    nc.scalar.activation(
                        out=h1s, in_=h1s, func=AF.Gelu, bias=sc["b1"][:],
                    )
                    # token-major table blocks via PE transpose + DVE copy
                    for s in range(2):
                        ptab = psTabp.tile([128, 4, C_MID], bf16, tag="ptab")
                        for jj in range(4):
                            c0 = p * SUB + 128 * jj
                            nc.tensor.transpose(
                                out=ptab[:, jj, :],
                                in_=h1cm[64 * s:64 * s + 64, c0:c0 + 128],
                                identity=identb_sb[64 * s:64 * s + 64,
                                                   64 * s:64 * s + 64],
                            )
                        b0 = 8 * p + 4 * s
                        tslice = table[:, 128 * b0:128 * b0 + 64]
                        nc.vector.tensor_copy(
                            out=bass.AP(
                                tensor=tslice.tensor, offset=tslice.offset,
                                ap=[[tslice.ap[0][0], 128], [128, 4],
                                    [1, C_MID]],
                            ),
                            in_=ptab[:],
                        )

        # ============ PHASE B/C: conv2 taps + epilogue (interleaved) =========
        acc_lo = accp.tile([128, 2, NGLO, C_MID], bf16)
        nc.vector.memset(acc_lo[:], 0.0)
        acc_hi = accp.tile([128, 2, NGHI, C_MID], bf16)
        nc.vector.memset(acc_hi[:], 0.0)
        nlo_reg = nc.gpsimd.to_reg(NKLO)
        nhi_reg = nc.gpsimd.to_reg(NKHI)
        nmini_reg = nc.gpsimd.to_reg(NMINI)
        zreg = nc.gpsimd.to_reg(0)

        gi_all = accp.tile([128, 8, (NKLO + NKHI) // 16], i16)
        nc.sync.dma_start(out=gi_all[:], in_=gidx[:].rearrange("k p f -> p k f"))
        si_all = accp.tile([128, 8, (NKLO + NKHI) // 16], i16)
        nc.sync.dma_start(out=si_all[:], in_=sidx[:].rearrange("k p f -> p k f"))

        with tc.tile_pool(name="gath", bufs=3) as gp, \
             tc.tile_pool(name="maskp", bufs=2) as mkp, \
             tc.tile_pool(name="h1mp", bufs=2) as h1mp, \
             tc.tile_pool(name="ssrc", bufs=2) as sp, \
             tc.tile_pool(name="h2fp", bufs=2) as h2fp, \
             tc.tile_pool(name="h2np", bufs=2) as h2p, \
             tc.tile_pool(name="rb3p", bufs=2) as rb3p, \
             tc.tile_pool(name="ysb", bufs=2) as yp, \
             tc.tile_pool(name="yout", bufs=2) as yop, \
             tc.tile_pool(name="psB", bufs=1, space="PSUM") as psB, \
             tc.tile_pool(name="psT", bufs=2, space="PSUM") as psTp, \
             tc.tile_pool(name="psC2", bufs=1, space="PSUM") as psC2p, \
             tc.tile_pool(name="psGC", bufs=1, space="PSUM") as psGC, \
             tc.tile_pool(name="psU", bufs=1, space="PSUM") as psUp, \
             tc.tile_pool(name="psC3", bufs=2, space="PSUM") as psC3:

            def _params(k8, half):
                if half == 0:
                    nk, nch, off16, nreg, acc = NKLO, NCHLO, 0, nlo_reg, acc_lo
                else:
                    nk, nch, off16, nreg, acc = (
                        NKHI, NCHHI, NKLO // 16, nhi_reg, acc_hi
                    )
                if k8 >= 6:  # boundary minis
                    nk, nch, nreg = NMINI, 1, nmini_reg
                return nk, nch, off16, nreg, acc

            def emit_gather(k8, half):
                nk, nch, off16, nreg, acc = _params(k8, half)
                gi = gi_all[:, k8, off16:off16 + nk // 16]
                gath = gp.tile([128, 1, NKHI], bf16, tag="gath")
                nc.gpsimd.dma_gather(
                    out_ap=gath[:, :, 0:nk],
                    in_ap=table[:],
                    idxs_ap=gi,
                    num_idxs=nk,
                    num_idxs_reg=nreg,
                    elem_size=128,
                    transpose=True,
                    sbuf_tokens_per_rank=128,
                    sbuf_free_dim_per_rank=256,
                    sbuf_free_dim_pad_per_rank=0,
                    sbuf_byte_offset=0,
                    single_packet=False,
                )
                return gath

            def emit_tap(k8, k, half, gath):
                nk, nch, off16, nreg, acc = _params(k8, half)
                si = si_all[:, k8, off16:off16 + nk // 16]
                src = sp.tile([128, NCHHI, C_MID], bf16, tag="src")
                for gch in range(0, nch, 8):
                    gg = min(8, nch - gch)
                    psb = psB.tile([128, SUB], f32, tag="psb")
                    for j in range(gg):
                        ch = gch + j
                        nc.tensor.matmul(
                            out=psb[:, 64 * j:64 * (j + 1)],
                            lhsT=gath[0:64, 0, 128 * ch:128 * (ch + 1)],
                            rhs=w2_sb[:, k, :],
                            start=True,
                            stop=True,
                        )
                    nc.vector.tensor_copy(
                        out=src[:, gch:gch + gg, :].rearrange("p a b -> p (a b)"),
                        in_=psb[:, 0:64 * gg],
                    )
                nc.gpsimd.dma_scatter_add(
                    acc[:, 0, :, :],
                    src[:, 0:nch, :],
                    si,
                    nk,
                    nreg,
                    C_MID,
                    parity_reg=zreg,
                    out_ap_other=acc[:, 1, :, :],
                    sbuf_tokens_per_rank=128,
                    single_packet=False,
                )

            def emit_epi_group(g):
                ps = pairs_of(g)
                gridB = psGC.tile([128, SUB], f32, tag="grid")
                if len(ps) == 1:
                    nc.vector.memset(gridB[0:4, :], 1.0)
                    nc.vector.memset(gridB[64:68, :], 1.0)
                h2fs = {}
                for j, p in enumerate(ps):
                    # neighbor sums: transpose accumulator blocks to ch-major
                    psT = psTp.tile([64, 2, SUB], bf16, tag="psT")
                    for s in range(2):
                        for jj in range(4):
                            b = 8 * p + 4 * s + jj
                            if b < 96:
                                blk = acc_lo[:, b & 1, b >> 1, :]
                            else:
                                b2 = b - 96
                                blk = acc_hi[:, b2 & 1, b2 >> 1, :]
                            nc.tensor.transpose(
                                out=psT[:, s, 128 * jj:128 * (jj + 1)],
                                in_=blk,
                                identity=identb_sb[:],
                            )
                    # masked shifted copies of h1 for the same-row taps
                    mt = mkp.tile([128, 2, SUB], bf16, tag="mk")
                    nc.sync.dma_start(out=mt[:], in_=mask_d[:, p, :, :])
                    h1m3 = h1mp.tile([128, SUB], bf16, tag="m3")
                    nc.vector.scalar_tensor_tensor(
                        out=h1m3[:], in0=h1cm[:, p * SUB:(p + 1) * SUB],
                        scalar=ones_sb[:], in1=mt[:, 0, :],
                        op0=OP.mult, op1=OP.mult,
                    )
                    h1m5 = h1mp.tile([128, SUB], bf16, tag="m5")
                    nc.vector.scalar_tensor_tensor(
                        out=h1m5[:], in0=h1cm[:, p * SUB:(p + 1) * SUB],
                        scalar=ones_sb[:], in1=mt[:, 1, :],
                        op0=OP.mult, op1=OP.mult,
                    )
                    # center + same-row taps accumulate in one PSUM bank
                    psC2 = psC2p.tile([128, SUB], f32, tag="psC2")
                    for s in range(2):
                        nc.tensor.matmul(
                            out=psC2[64 * s:64 * s + 64, :],
                            lhsT=w2c_sb[64 * s:64 * s + 64, :],
                            rhs=h1cm[64 * s:64 * s + 64,
                                     p * SUB:(p + 1) * SUB],
                            start=True,
                            stop=False,
                            tile_position=(64 * s, 64 * s),
                        )
                        nc.tensor.matmul(
                            out=psC2[64 * s:64 * s + 64, 1:SUB],
                            lhsT=w2d_sb[64 * s:64 * s + 64, 0, :],
                            rhs=h1m3[64 * s:64 * s + 64, 0:SUB - 1],
                            start=False,
                            stop=False,
                            tile_position=(64 * s, 64 * s),
                        )
                        nc.tensor.matmul(
                            out=psC2[64 * s:64 * s + 64, 0:SUB - 1],
                            lhsT=w2d_sb[64 * s:64 * s + 64, 1, :],
                            rhs=h1m5[64 * s:64 * s + 64, 1:SUB],
                            start=False,
                            stop=True,
                            tile_position=(64 * s, 64 * s),
                        )
                    h2f = h2fp.tile([128, SUB], f32, tag="h2f")
                    for s in range(2):
                        nc.vector.tensor_copy(
                            out=h2f[64 * s:64 * s + 64, :],
                            in_=psT[:, s, :],
                        )
                    nc.vector.scalar_tensor_tensor(
                        out=h2f[:], in0=psC2[:], scalar=ones_sb[:],
                        in1=h2f[:], op0=OP.mult, op1=OP.add,
                    )
                    h2fs[j] = h2f
                    sq = sqp.tile([128, SUB], f32r, tag="sq")
                    nc.vector.scalar_tensor_tensor(
                        out=sq[:], in0=h2f[:], scalar=ones_sb[:],
                        in1=h2f[:], op0=OP.mult, op1=OP.mult,
                    )
                    nc.tensor.matmul(
                        out=gridB[0:4, :],
                        lhsT=es_sb[:, j, :],
                        rhs=sq[:],
                        start=(j == 0),
                        stop=(j == len(ps) - 1),
                    )
                rstd2 = rsp.tile([4, SUB], f32, tag="rstd")
                nc.scalar.activation(
                    out=rstd2[:], in_=gridB[0:4, :], func=AF.Sqrt,
                    bias=eps_sb[0:4, :], scale=1.0 / C_MID,
                )
                nc.vector.reciprocal(out=rstd2[:], in_=rstd2[:])
                rsc2 = drp.tile([4, SUB], f32, tag="rsc")
                nc.sync.dma_start(out=rsc2[:], in_=rstd2[:])
                rb2 = rbp.tile([128, 2, SUB], f32, tag="rb1")
                for j in range(len(ps)):
                    for s in range(2):
                        nc.sync.dma_start(
                            out=rb2[64 * s:64 * s + 64, j, :],
                            in_=_bcast_ap(
                                rsc2[2 * j + s:2 * j + s + 1, :],
                                [[0, 64], [1, SUB]],
                            ),
                        )
                h2ns = {}
                for j, p in enumerate(ps):
                    h2n = h2p.tile([128, SUB], bf16, tag="h2n")
                    h2ns[j] = h2n
                    nc.vector.scalar_tensor_tensor(
                        out=h2n[:], in0=h2fs[j][:], scalar=sc["g2"][:],
                        in1=rb2[:, j, :], op0=OP.mult, op1=OP.mult,
                    )
                    nc.scalar.activation(
                        out=h2n[:], in_=h2n[:], func=AF.Gelu, bias=sc["b2"][:],
                    )
                    u = psUp.tile([128, SUB], f32, tag="U")
                    for s in range(2):
                        nc.tensor.matmul(
                            out=u[64 * s:64 * s + 64, :],
                            lhsT=l_sb[64 * s:64 * s + 64, :],
                            rhs=h2n[64 * s:64 * s + 64, :],
                            start=True,
                            stop=True,
                            tile_position=(64 * s, 64 * s),
                        )
                    squ = sqp.tile([128, SUB], bf16, tag="squ")
                    nc.scalar.activation(out=squ[:], in_=u[:], func=AF.Square)
                    nc.tensor.matmul(
                        out=gridB[64:68, :],
                        lhsT=esb_sb[:, j, :],
                        rhs=squ[:],
                        start=(j == 0),
                        stop=(j == len(ps) - 1),
                        tile_position=(0, 64),
                    )
                rstd3 = rsp.tile([4, SUB], f32, tag="rstd3")
                nc.scalar.activation(
                    out=rstd3[:], in_=gridB[64:68, :], func=AF.Sqrt,
                    bias=eps_sb[0:4, :], scale=1.0,
                )
                nc.vector.reciprocal(out=rstd3[:], in_=rstd3[:])
                rsc3 = drp.tile([4, SUB], f32, tag="rsc")
                nc.sync.dma_start(out=rsc3[:], in_=rstd3[:])
                rb3 = rb3p.tile([128, 2, 2, SUB], f32, tag="rb3")
                for j in range(len(ps)):
                    for s in range(2):
                        nc.sync.dma_start(
                            out=rb3[:, j, s, :],
                            in_=_bcast_ap(
                                rsc3[2 * j + s:2 * j + s + 1, :],
                                [[0, 128], [1, SUB]],
                            ),
                        )
                for j, p in enumerate(ps):
                    x2 = xp.tile([128, 2, PAIR], bf16, tag="xres")
                    nc.scalar.dma_start(
                        out=x2[:],
                        in_=x_t[:, :, p * PAIR:(p + 1) * PAIR].rearrange(
                            "c p f -> p c f"
                        ),
                    )
                    for c in range(2):
                        y_sb = yp.tile([128, PAIR], f32, tag="y")
                        for s in range(2):
                            a3 = psC3.tile([128, SUB], f32, tag="A3")
                            nc.tensor.matmul(
                                out=a3[:],
                                lhsT=w3_sb[64 * s:64 * s + 64,
                                           128 * c:128 * (c + 1)],
                                rhs=h2ns[j][64 * s:64 * s + 64, :],
                                start=True,
                                stop=True,
                                tile_position=(64 * s, 0),
                            )
                            nc.vector.scalar_tensor_tensor(
                                out=y_sb[:, SUB * s:SUB * (s + 1)],
                                in0=a3[:], scalar=g3_sb[:, c:c + 1],
                                in1=rb3[:, j, s, :], op0=OP.mult, op1=OP.mult,
                            )
                        yout = yop.tile([128, PAIR], bf16, tag="yo")
                        nc.vector.scalar_tensor_tensor(
                            out=y_sb[:], in0=y_sb[:],
                            scalar=ones_sb[:],
                            in1=x2[:, c, :], op0=OP.mult, op1=OP.add,
                        )
                        nc.scalar.activation(
                            out=yout[:], in_=y_sb[:],
                            func=AF.Gelu, bias=b3_sb[:, c:c + 1],
                        )
                        nc.scalar.dma_start(
                            out=y_t[c, :, p * PAIR:(p + 1) * PAIR],
                            in_=yout[:],
                        )

            # schedule: (half, tap) steps; gathers issued 2 steps ahead
            steps = [(0, k8) for k8 in range(8)] + [(1, k8) for k8 in range(8)]
            gaths = {}
            for i in range(len(steps) + 2):
                if i < len(steps):
                    h, k8 = steps[i]
                    gaths[i] = emit_gather(k8, h)
                j = i - 2
                if 0 <= j < len(steps):
                    h, k8 = steps[j]
                    emit_tap(k8, KTAP[k8], h, gaths.pop(j))
                    if h == 0 and k8 == 7 and dbg:
                        nc.sync.dma_start(out=acclo_out[:], in_=acc_lo[:])
                    # interleave lo epilogue groups among hi taps
                    if h == 1 and k8 in (1, 2, 3, 4, 5, 6):
                        emit_epi_group(k8 - 1)
            if dbg:
                nc.sync.dma_start(out=acchi_out[:], in_=acc_hi[:])
            for g in range(6, NGRP):
                emit_epi_group(g)

    nc.compile()
    _NC_CACHE["nc"] = nc
    return nc


# ======================= host-side sharding =======================

def _components(nbr):
    """Connected-component labels via vectorized min-label propagation."""
    lab = np.arange(N, dtype=np.int64)
    ks = [k for k in range(9) if k != 4]
    valid = [(nbr[k] < N) for k in ks]
    nbrs = [nbr[k].astype(np.int64) for k in ks]
    for _ in range(200):
        new = lab.copy()
        for k in range(len(ks)):
            v = valid[k]
            cand = lab[nbrs[k][v]]
            np.minimum.at(new, np.nonzero(v)[0], cand)
        new = np.minimum(new, new[new])
        if np.array_equal(new, lab):
            break
        lab = new
    while True:
        new = lab[lab]
        if np.array_equal(new, lab):
            break
        lab = new
    return lab


def _shard(nbr):
    lab = _components(nbr)
    comp_ids, comp_inv, comp_sizes = np.unique(
        lab, return_inverse=True, return_counts=True
    )
    order = np.argsort(comp_sizes)[::-1]
    import heapq

    heap = [(0, c) for c in range(NCORES)]
    heapq.heapify(heap)
    comp_core = np.empty(len(comp_ids), dtype=np.int64)
    for ci in order:
        load, core = heapq.heappop(heap)
        comp_core[ci] = core
        heapq.heappush(heap, (load + int(comp_sizes[ci]), core))
    point_core = comp_core[comp_inv]
    ids_per_core = [np.nonzero(point_core == c)[0] for c in range(NCORES)]
    for c in range(NCORES):
        assert len(ids_per_core[c]) <= T, f"core {c} overloaded: {len(ids_per_core[c])}"
    return ids_per_core


def _wrap_idx(flat):
    """[n] int16 -> [128, n//16] (16-wrapped, replicated x8)."""
    n = flat.shape[0]
    w = flat.reshape(n // 16, 16).T
    return np.ascontiguousarray(np.tile(w, (8, 1)))


def _order_runs(nbr, ids):
    """Order a core's tokens along horizontal runs so the (0,-1)/(0,+1)
    neighbors become local ids t-1/t+1."""
    ids = np.asarray(ids)
    inset = np.zeros(N + 1, dtype=bool)
    inset[ids] = True
    left = nbr[3, ids].astype(np.int64)
    has_left = (left < N) & inset[np.clip(left, 0, N)]
    starts = ids[~has_left]
    right = np.full(N + 1, N, dtype=np.int64)
    r = nbr[5, ids].astype(np.int64)
    ok = (r < N) & inset[np.clip(r, 0, N)]
    right[ids[ok]] = r[ok]
    out = np.empty(len(ids), dtype=np.int64)
    pos = 0
    for s in starts:
        cur = int(s)
        while cur != N:
            out[pos] = cur
            pos += 1
            cur = int(right[cur])
    assert pos == len(ids)
    return out


def _prep_core(x, nbr, ids):
    import ml_dtypes
    ids = _order_runs(nbr, ids)
    n = len(ids)
    glob2loc = np.full(N + 1, SENT, dtype=np.int64)
    glob2loc[ids] = np.arange(n)
    xl = np.zeros((T, C_IN), dtype=np.float32)
    xl[:n] = x[ids]
    x_t = np.ascontiguousarray(
        xl.T.reshape(2, 128, T).astype(ml_dtypes.bfloat16)
    )
    nbl = np.full((9, T), SENT, dtype=np.int64)
    nbl[:, :n] = glob2loc[np.where(nbr[:, ids] < N, nbr[:, ids], N)]
    gidx = np.zeros((8, 128, (NKLO + NKHI) // 16), dtype=np.int16)
    sidx = np.zeros((8, 128, (NKLO + NKHI) // 16), dtype=np.int16)

    def fill(k8, dst, srcl):
        lo = dst < SPLIT
        dlo, slo = dst[lo], srcl[lo]
        dhi, shi = dst[~lo] - SPLIT, srcl[~lo]
        nkl = NKLO if k8 < 6 else NMINI
        nkh = NKHI if k8 < 6 else NMINI
        assert len(dlo) <= nkl, f"tap {k8}: {len(dlo)} lo edges > {nkl}"
        assert len(dhi) <= nkh, f"tap {k8}: {len(dhi)} hi edges > {nkh}"
        gf = np.full(NKLO + NKHI, SENT, dtype=np.int16)
        gf[:len(slo)] = slo.astype(np.int16)
        gf[NKLO:NKLO + len(shi)] = shi.astype(np.int16)
        sf = np.empty(NKLO + NKHI, dtype=np.int16)
        sf[:NKLO] = LO_TRASH
        sf[:len(dlo)] = dlo.astype(np.int16)
        sf[NKLO:] = HI_TRASH
        sf[NKLO:NKLO + len(dhi)] = dhi.astype(np.int16)
        gidx[k8] = np.concatenate(
            [_wrap_idx(gf[:NKLO]), _wrap_idx(gf[NKLO:])], axis=1
        )
        sidx[k8] = np.concatenate(
            [_wrap_idx(sf[:NKLO]), _wrap_idx(sf[NKLO:])], axis=1
        )

    for k8, k in enumerate([0, 1, 2, 6, 7, 8]):
        dst = np.nonzero(nbl[k, :n] != SENT)[0]
        fill(k8, dst, nbl[k, dst])
    # same-row taps: shift+mask path; only subtile-boundary edges gathered
    mask = np.zeros((2, T), dtype=np.float32)
    for mi, k in enumerate((3, 5)):
        dst = np.nonzero(nbl[k, :n] != SENT)[0]
        srcl = nbl[k, dst]
        step = -1 if k == 3 else 1
        assert np.all(srcl == dst + step), f"tap {k} not shift-structured"
        bnd = (dst % SUB == 0) if k == 3 else (dst % SUB == SUB - 1)
        fill(6 + mi, dst[bnd], srcl[bnd])
        ok = ~bnd
        mask[mi, srcl[ok]] = 1.0
    import ml_dtypes
    mrs = mask.reshape(2, NPAIR, 2, SUB)
    mask_d = np.ascontiguousarray(
        np.broadcast_to(
            np.stack([mrs[0], mrs[1]], axis=2)[None],
            (128, NPAIR, 2, 2, SUB),
        ).reshape(128, NPAIR, 2 * 2, SUB)[:, :, [0, 3], :]
    )
    # mask tile layout: [128 part, pair, tap, col]; partition 64s+c needs the
    # mask of token (p, s, col) -> build per-sub then select diag entries
    m2 = np.empty((128, NPAIR, 2, SUB), dtype=np.float32)
    for s in range(2):
        for mi in range(2):
            m2[64 * s:64 * s + 64, :, mi, :] = mrs[mi, :, s, :][None]
    mask_d = np.ascontiguousarray(m2.astype(ml_dtypes.bfloat16))
    return x_t, gidx, sidx, mask_d, ids, n


def _prep_weights(W1, W2, W3, g1, b1, g2, b2, g3, b3):
    import ml_dtypes
    W1 = np.asarray(W1, np.float64)
    W2 = np.asarray(W2, np.float64)
    W3 = np.asarray(W3, np.float64)

    def center(w, C):
        return w - w.mean(axis=-1, keepdims=True)

    W1c = center(W1, C_MID)          # [256, 64]
    W2cc = center(W2, C_MID)         # [9, 64, 64]
    W3c = center(W3, C_IN)           # [64, 256]
    w1 = np.ascontiguousarray(
        W1c.reshape(2, 128, C_MID).transpose(1, 0, 2).astype(np.float32)
        .astype(ml_dtypes.bfloat16)
    )
    w2 = np.ascontiguousarray(
        W2cc.transpose(1, 0, 2).astype(np.float32).astype(ml_dtypes.bfloat16)
    )  # [64, 9, 64]
    w2c = np.ascontiguousarray(
        np.tile(W2cc[4].astype(np.float32), (2, 1)).astype(ml_dtypes.bfloat16)
    )  # [128, 64]
    w2d = np.ascontiguousarray(
        np.tile(
            np.stack([W2cc[3], W2cc[5]], axis=1).astype(np.float32), (2, 1, 1)
        ).astype(ml_dtypes.bfloat16)
    )  # [128, 2, 64]
    w3 = np.ascontiguousarray(
        np.tile(W3c.astype(np.float32), (2, 1)).astype(ml_dtypes.bfloat16)
    )  # [128, 256] rows 64-127 replicated
    M3 = (W3c @ W3c.T) / C_IN
    L = np.linalg.cholesky(M3 + 1e-12 * np.eye(C_MID))
    lmat = np.ascontiguousarray(
        np.tile(L.astype(np.float32), (2, 1)).astype(ml_dtypes.bfloat16)
    )  # [128, 64]

    def rep2(v):
        return np.ascontiguousarray(
            np.tile(np.asarray(v, np.float32).reshape(C_MID), 2).reshape(128, 1)
        )

    g1r, b1r, g2r, b2r = rep2(g1), rep2(b1), rep2(g2), rep2(b2)
    g3r = np.ascontiguousarray(np.asarray(g3, np.float32).reshape(2, 128).T)
    b3r = np.ascontiguousarray(np.asarray(b3, np.float32).reshape(2, 128).T)
    es = np.zeros((128, 2, 4), np.float32)
    for j in range(2):
        for p in range(128):
            es[p, j, 2 * j + p // 64] = 1.0
    identb = np.eye(128, dtype=np.float32).astype(ml_dtypes.bfloat16)
    return (w1, w2, w2c, w2d, w3, lmat, g1r, b1r, g2r, b2r, g3r, b3r, es,
            identb)


def prep_in_maps(inputs):
    x = np.asarray(inputs["x"], np.float32)
    nbr = np.asarray(inputs["neighbor_idx"])
    (w1, w2, w2c, w2d, w3, lmat, g1r, b1r, g2r, b2r, g3r, b3r, es,
     identb) = _prep_weights(
        inputs["W1"], inputs["W2"], inputs["W3"], inputs["g1"], inputs["b1"],
        inputs["g2"], inputs["b2"], inputs["g3"], inputs["b3"],
    )
    ids_per_core = _shard(nbr)
    in_maps = []
    metas = []
    for c in range(NCORES):
        x_t, gidx, sidx, mask_d, ids_o, n = _prep_core(x, nbr, ids_per_core[c])
        metas.append((ids_o, n))
        in_maps.append(
            dict(
                x_t=x_t, gidx=gidx, sidx=sidx, mask_d=mask_d, w1=w1, w2=w2,
                w2c=w2c, w2d=w2d, w3=w3,
                lmat=lmat, es=es, g1r=g1r, b1r=b1r, g2r=g2r, b2r=b2r,
                g3r=g3r, b3r=b3r, identb=identb,
            )
        )
    return in_maps, metas


def kernel(x, W1, W2, W3, g1, b1, g2, b2, g3, b3, neighbor_idx):
    in_maps, metas = prep_in_maps(
        dict(
            x=x, W1=W1, W2=W2, W3=W3, g1=g1, b1=b1, g2=g2, b2=b2,
            g3=g3, b3=b3, neighbor_idx=neighbor_idx,
        )
    )
    nc = build_nc()
    res = run_bass_kernel_spmd(nc, in_maps, core_ids=list(range(NCORES)))
    y = np.empty((N, C_IN), dtype=np.float32)
    for c in range(NCORES):
        yt = res.results[c]["y_t"]  # [2, 128, T] bf16
        ids, n = metas[c]
        yl = np.asarray(yt, dtype=np.float32).reshape(C_IN, T).T  # [T, 256]
        y[ids] = yl[:n]
    return y
